# revision 24
# baseline (speedup 1.0000x reference)
"""DCT-based 1.25x upsample (2D DCT-II -> zero-pad spectrum -> 2D IDCT).

The whole reference computation is linear per (b, c) slice:
    out = M @ x @ M^T,   M = E960[:, :768] @ D768  (960x768, precomputed on host)
where D is the DCT-II matrix (norm=None) and E the IDCT matrix; zero-padding
the spectrum is folded into taking the first 768 columns of E.

On each NeuronCore (data-parallel over batch: 16 batches / 8 cores = 2 each,
x 3 channels = 6 slices per core) we run two chained matmuls per slice with
the tensor engine's `out = lhsT.T @ rhs` convention and the shared constant
Mt = M^T (768x960) as the moving operand:
    step 1:  W   = matmul(lhsT=x,  rhs=Mt) = x^T @ M^T         [768, 960]
    step 2:  out = matmul(lhsT=W,  rhs=Mt) = M @ x @ M^T       [960, 960]
W comes out of step 1 in PSUM with exactly the [K-partition, free] layout
step 2 needs for lhsT, so no transposes anywhere.

Matmuls run as float32r (fp32 bits, FP22 multiplies, fp32 accumulate):
1 PE cycle/row vs 4 for true fp32, end-to-end rel err ~1e-4.
"""

import numpy as np

import concourse.bass as bass  # noqa: F401  (engine types route via nc)
import concourse.mybir as mybir
import concourse.tile as tile
from concourse import bacc
from concourse.bass_utils import run_bass_kernel_spmd

# Problem shape (hardcoded per contract)
B, C, H = 16, 3, 768
OUT = 960  # H * 1.25
N_CORES = 8
SLICES = (B * C) // N_CORES  # 6 per core

P = 128
KT = H // P          # 6 contraction tiles
MT1 = H // P         # 6 output-row tiles for step 1 (x columns)
M2 = 120
MT2 = OUT // M2      # 8 output-row tiles for step 2
NT = 2
NW = OUT // NT       # 480-wide moving chunks (<= 512 fp32 PSUM bank)

MM_DT = mybir.dt.float32r  # set to mybir.dt.float32 for full-precision (4x slower)


def _build_mt() -> np.ndarray:
    """Mt = (E960[:, :768] @ D768)^T as float32, computed in float64."""
    n = np.arange(H, dtype=np.float64)
    k = np.arange(H, dtype=np.float64)[:, None]
    D = 2.0 * np.cos(np.pi * (2.0 * n[None, :] + 1.0) * k / (2.0 * H))

    n2 = np.arange(OUT, dtype=np.float64)[:, None]
    k2 = np.arange(OUT, dtype=np.float64)[None, :]
    E = np.cos(np.pi * (2.0 * n2 + 1.0) * k2 / (2.0 * OUT)) / OUT
    E[:, 0] = 1.0 / (2.0 * OUT)

    M = E[:, :H] @ D  # [960, 768]
    return np.ascontiguousarray(M.T).astype(np.float32)  # [768, 960]


def _build_program():
    nc = bacc.Bacc(None, target_bir_lowering=False, debug=False)

    # Both inputs are pre-arranged on the host into the striped SBUF layout
    # (partition-major), so every load DMA is one contiguous run per partition.
    x_ext = nc.dram_tensor("x", [SLICES, P, KT * H], MM_DT, kind="ExternalInput")
    mt_ext = nc.dram_tensor("mt", [P, KT * OUT], MM_DT, kind="ExternalInput")
    out_ext = nc.dram_tensor(
        "out", [SLICES, OUT, OUT], mybir.dt.float32, kind="ExternalOutput"
    )

    with tile.TileContext(nc) as tc:
        with (
            tc.tile_pool(name="const", bufs=1) as const_pool,
            tc.tile_pool(name="xp", bufs=4) as x_pool,
            tc.tile_pool(name="wp", bufs=2) as w_pool,
            tc.tile_pool(name="op", bufs=4) as o_pool,
            tc.tile_pool(name="ps", bufs=8, space="PSUM") as psum_pool,
        ):
            # PE warmup: ~16 dummy matmuls on memset tiles keep the tensor
            # engine busy while the first loads land, so the HAM clock gate is
            # already at 2.4 GHz when real matmuls start.
            warm_f32 = const_pool.tile([P, NW], mybir.dt.float32, name="warm_f32")
            nc.gpsimd.memset(warm_f32[:], 0.0)
            warm_w = const_pool.tile([P, P], MM_DT, name="warm_w")
            warm_m = const_pool.tile([P, NW], MM_DT, name="warm_m")
            nc.vector.tensor_copy(warm_w[:], warm_f32[:, :P])
            nc.scalar.copy(warm_m[:], warm_f32[:])
            warm_ps = psum_pool.tile([P, NW], mybir.dt.float32, tag="ps", name="warm_ps")
            for _ in range(20):
                nc.tensor.matmul(warm_ps[:], warm_w[:], warm_m[:], start=True, stop=True)

            # Shared constant Mt split into two K-group tiles (contiguous DRAM
            # runs per partition); the first matmuls only wait on group 0.
            # mt_kg[g][p, kl, n] = Mt[(g*KH + kl)*P + p, n]
            KH = KT // 2
            mt_dram = mt_ext[:].rearrange("p (ko n) -> p ko n", n=OUT)
            mt_kg = [
                const_pool.tile([P, KH, OUT], MM_DT, name=f"mt{g}") for g in range(2)
            ]
            # First-needed loads go on the Activation HWDGE queue, whose engine
            # preamble finishes ~2us before SP's -- and the SP queue's later
            # start naturally deprioritizes the second K-group transfers.
            nc.scalar.dma_start(mt_kg[0][:], mt_dram[:, 0:KH, :])

            for s in range(SLICES):
                # x slice split into two K-group (row) tiles:
                # x_kg[g][p, kl, j] = x[(g*KH + kl)*P + p, j]
                x_dram = x_ext[s].rearrange("p (ko j) -> p ko j", j=H)
                x_kg = []
                for g in range(2):
                    xg = x_pool.tile([P, KH, H], MM_DT, tag="x", name=f"x{g}")
                    dma_eng = nc.scalar if (s == 0 and g == 0) else nc.sync
                    dma_eng.dma_start(xg[:], x_dram[:, g * KH : (g + 1) * KH, :])
                    x_kg.append(xg)
                    if s == 0 and g == 0:
                        nc.sync.dma_start(mt_kg[1][:], mt_dram[:, KH:KT, :])

                # Step 1: W = x^T @ Mt, K-striped for step 2:
                # w_sb[p, m, l] = W[m*P + p, l]
                # k-outer over halves of m so first-slice matmuls start as soon
                # as stripe k=0 lands (6 live PSUM banks per half).
                w_sb = w_pool.tile([P, KT, OUT], MM_DT)
                MH = MT1 // 2
                if s == 0:
                    # Two-pass accumulation, K-group outer: all of group 0's
                    # matmuls run while group 1's loads are still in flight;
                    # partials stash in SBUF and fold back in during group 1.
                    w_part = w_pool.tile(
                        [P, KT, OUT], mybir.dt.float32, name="w_part", bufs=1
                    )
                    for g in range(2):
                        for half in range(2):
                            psums = [
                                [
                                    psum_pool.tile(
                                        [P, NW],
                                        mybir.dt.float32,
                                        tag="ps",
                                        name=f"ps{ml}_{n}",
                                    )
                                    for n in range(NT)
                                ]
                                for ml in range(MH)
                            ]
                            for kl in range(KH):
                                for ml in range(MH):
                                    m = half * MH + ml
                                    for n in range(NT):
                                        nc.tensor.matmul(
                                            psums[ml][n][:],
                                            x_kg[g][:, kl, m * P : (m + 1) * P],
                                            mt_kg[g][:, kl, n * NW : (n + 1) * NW],
                                            start=(kl == 0),
                                            stop=(kl == KH - 1),
                                        )
                            for ml in range(MH):
                                m = half * MH + ml
                                for n in range(NT):
                                    dst = slice(n * NW, (n + 1) * NW)
                                    if g == 0:
                                        nc.vector.tensor_copy(
                                            w_part[:, m, dst], psums[ml][n][:]
                                        )
                                    else:
                                        nc.vector.tensor_add(
                                            out=w_sb[:, m, dst],
                                            in0=psums[ml][n][:],
                                            in1=w_part[:, m, dst],
                                        )
                else:
                    for half in range(2):
                        psums = [
                            [
                                psum_pool.tile(
                                    [P, NW],
                                    mybir.dt.float32,
                                    tag="ps",
                                    name=f"ps{ml}_{n}",
                                )
                                for n in range(NT)
                            ]
                            for ml in range(MH)
                        ]
                        for k in range(KT):
                            g, kl = divmod(k, KH)
                            for ml in range(MH):
                                m = half * MH + ml
                                for n in range(NT):
                                    nc.tensor.matmul(
                                        psums[ml][n][:],
                                        x_kg[g][:, kl, m * P : (m + 1) * P],
                                        mt_kg[g][:, kl, n * NW : (n + 1) * NW],
                                        start=(k == 0),
                                        stop=(k == KT - 1),
                                    )
                        for ml in range(MH):
                            m = half * MH + ml
                            for n in range(NT):
                                nc.vector.tensor_copy(
                                    w_sb[:, m, n * NW : (n + 1) * NW], psums[ml][n][:]
                                )
                w_r = w_sb[:]

                # Step 2: out = W^T @ Mt
                for m in range(MT2):
                    psums = [
                        psum_pool.tile([P, NW], mybir.dt.float32, tag="ps", name=f"ps{n}")
                        for n in range(NT)
                    ]
                    o_sb = o_pool.tile([M2, OUT], mybir.dt.float32)
                    for k in range(KT):
                        g, kl = divmod(k, KH)
                        for n in range(NT):
                            nc.tensor.matmul(
                                psums[n][:M2, :],
                                w_r[:, k, m * M2 : (m + 1) * M2],
                                mt_kg[g][:, kl, n * NW : (n + 1) * NW],
                                start=(k == 0),
                                stop=(k == KT - 1),
                            )
                    # Finer evict/DMA chunks on the very last group shorten the
                    # kernel tail (the final DMA starts sooner).
                    ev_chunks = 4 if (s == SLICES - 1 and m == MT2 - 1) else 1
                    for n in range(NT):
                        cw = NW // ev_chunks
                        for c in range(ev_chunks):
                            lo = n * NW + c * cw
                            nc.vector.tensor_copy(
                                o_sb[:, lo : lo + cw], psums[n][:M2, c * cw : (c + 1) * cw]
                            )
                            nc.sync.dma_start(
                                out_ext[s, m * M2 : (m + 1) * M2, lo : lo + cw],
                                o_sb[:, lo : lo + cw],
                            )

    nc.compile()
    return nc


_CACHE: dict = {}


def _get_program():
    if "nc" not in _CACHE:
        _CACHE["nc"] = _build_program()
        _CACHE["mt"] = _build_mt()
    return _CACHE["nc"], _CACHE["mt"]


def kernel(x: np.ndarray, _trace: bool = False):
    assert x.shape == (B, C, H, H), x.shape
    nc, mt = _get_program()
    x = np.ascontiguousarray(x, dtype=np.float32)
    # Pre-stripe on host: rows -> (ko, p) partitions, contiguous per partition.
    mt_arr = np.ascontiguousarray(
        mt.reshape(KT, P, OUT).transpose(1, 0, 2).reshape(P, KT * OUT)
    )
    x_arr = np.ascontiguousarray(
        x.reshape(B * C, KT, P, H).transpose(0, 2, 1, 3).reshape(B * C, P, KT * H)
    )
    per_core = B // N_CORES
    in_maps = [
        {
            "x": x_arr[i * SLICES : (i + 1) * SLICES],
            "mt": mt_arr,
        }
        for i in range(N_CORES)
    ]
    res = run_bass_kernel_spmd(nc, in_maps, list(range(N_CORES)), trace=_trace)
    out = np.empty((B, C, OUT, OUT), dtype=np.float32)
    for i in range(N_CORES):
        out[i * per_core : (i + 1) * per_core] = res.results[i]["out"].reshape(
            per_core, C, OUT, OUT
        )
    if _trace:
        return out, res
    return out


# revision 25
# speedup vs baseline: 1.0520x; 1.0520x over previous
"""DCT-based 1.25x upsample (2D DCT-II -> zero-pad spectrum -> 2D IDCT).

The whole reference computation is linear per (b, c) slice:
    out = M @ x @ M^T,   M = E960[:, :768] @ D768  (960x768, precomputed on host)
where D is the DCT-II matrix (norm=None) and E the IDCT matrix; zero-padding
the spectrum is folded into taking the first 768 columns of E.

On each NeuronCore (data-parallel over batch: 16 batches / 8 cores = 2 each,
x 3 channels = 6 slices per core) we run two chained matmuls per slice with
the tensor engine's `out = lhsT.T @ rhs` convention and the shared constant
Mt = M^T (768x960) as the moving operand:
    step 1:  W   = matmul(lhsT=x,  rhs=Mt) = x^T @ M^T         [768, 960]
    step 2:  out = matmul(lhsT=W,  rhs=Mt) = M @ x @ M^T       [960, 960]
W comes out of step 1 in PSUM with exactly the [K-partition, free] layout
step 2 needs for lhsT, so no transposes anywhere.

Matmuls run as float32r (fp32 bits, FP22 multiplies, fp32 accumulate):
1 PE cycle/row vs 4 for true fp32, end-to-end rel err ~1e-4.
"""

import numpy as np

import concourse.bass as bass  # noqa: F401  (engine types route via nc)
import concourse.mybir as mybir
import concourse.tile as tile
from concourse import bacc
from concourse.bass_utils import run_bass_kernel_spmd

# Problem shape (hardcoded per contract)
B, C, H = 16, 3, 768
OUT = 960  # H * 1.25
N_CORES = 8
SLICES = (B * C) // N_CORES  # 6 per core

P = 128
KT = H // P          # 6 contraction tiles
MT1 = H // P         # 6 output-row tiles for step 1 (x columns)
M2 = 120
MT2 = OUT // M2      # 8 output-row tiles for step 2
NT = 2
NW = OUT // NT       # 480-wide moving chunks (<= 512 fp32 PSUM bank)

MM_DT = mybir.dt.float32r  # set to mybir.dt.float32 for full-precision (4x slower)


def _build_mt() -> np.ndarray:
    """Mt = (E960[:, :768] @ D768)^T as float32, computed in float64."""
    n = np.arange(H, dtype=np.float64)
    k = np.arange(H, dtype=np.float64)[:, None]
    D = 2.0 * np.cos(np.pi * (2.0 * n[None, :] + 1.0) * k / (2.0 * H))

    n2 = np.arange(OUT, dtype=np.float64)[:, None]
    k2 = np.arange(OUT, dtype=np.float64)[None, :]
    E = np.cos(np.pi * (2.0 * n2 + 1.0) * k2 / (2.0 * OUT)) / OUT
    E[:, 0] = 1.0 / (2.0 * OUT)

    M = E[:, :H] @ D  # [960, 768]
    return np.ascontiguousarray(M.T).astype(np.float32)  # [768, 960]


def _build_program():
    nc = bacc.Bacc(None, target_bir_lowering=False, debug=False)

    # Both inputs are pre-arranged on the host into the striped SBUF layout
    # (partition-major), so every load DMA is one contiguous run per partition.
    x_ext = nc.dram_tensor("x", [SLICES, P, KT * H], MM_DT, kind="ExternalInput")
    mt_ext = nc.dram_tensor("mt", [P, KT * OUT], MM_DT, kind="ExternalInput")
    out_ext = nc.dram_tensor(
        "out", [SLICES, OUT, OUT], mybir.dt.float32, kind="ExternalOutput"
    )

    with tile.TileContext(nc) as tc:
        with (
            tc.tile_pool(name="const", bufs=1) as const_pool,
            tc.tile_pool(name="xp", bufs=4) as x_pool,
            tc.tile_pool(name="wp", bufs=2) as w_pool,
            tc.tile_pool(name="op", bufs=4) as o_pool,
            tc.tile_pool(name="ps", bufs=8, space="PSUM") as psum_pool,
        ):
            # PE warmup: ~16 dummy matmuls on memset tiles keep the tensor
            # engine busy while the first loads land, so the HAM clock gate is
            # already at 2.4 GHz when real matmuls start.
            warm_f32 = const_pool.tile([P, NW], mybir.dt.float32, name="warm_f32")
            nc.gpsimd.memset(warm_f32[:], 0.0)
            warm_w = const_pool.tile([P, P], MM_DT, name="warm_w")
            warm_m = const_pool.tile([P, NW], MM_DT, name="warm_m")
            nc.vector.tensor_copy(warm_w[:], warm_f32[:, :P])
            nc.scalar.copy(warm_m[:], warm_f32[:])
            warm_ps = psum_pool.tile([P, NW], mybir.dt.float32, tag="ps", name="warm_ps")
            for _ in range(20):
                nc.tensor.matmul(warm_ps[:], warm_w[:], warm_m[:], start=True, stop=True)

            # Shared constant Mt split into two K-group tiles (contiguous DRAM
            # runs per partition); the first matmuls only wait on group 0.
            # mt_kg[g][p, kl, n] = Mt[(g*KH + kl)*P + p, n]
            KH = KT // 2
            mt_dram = mt_ext[:].rearrange("p (ko n) -> p ko n", n=OUT)
            mt_kg = [
                const_pool.tile([P, KH, OUT], MM_DT, name=f"mt{g}") for g in range(2)
            ]
            nc.sync.dma_start(mt_kg[0][:], mt_dram[:, 0:KH, :])

            for s in range(SLICES):
                # x slice split into two K-group (row) tiles:
                # x_kg[g][p, kl, j] = x[(g*KH + kl)*P + p, j]
                x_dram = x_ext[s].rearrange("p (ko j) -> p ko j", j=H)
                x_kg = []
                for g in range(2):
                    xg = x_pool.tile([P, KH, H], MM_DT, tag="x", name=f"x{g}")
                    nc.sync.dma_start(xg[:], x_dram[:, g * KH : (g + 1) * KH, :])
                    x_kg.append(xg)
                    if s == 0 and g == 0:
                        nc.sync.dma_start(mt_kg[1][:], mt_dram[:, KH:KT, :])

                # Step 1: W = x^T @ Mt, K-striped for step 2:
                # w_sb[p, m, l] = W[m*P + p, l]
                # k-outer over halves of m so first-slice matmuls start as soon
                # as stripe k=0 lands (6 live PSUM banks per half).
                w_sb = w_pool.tile([P, KT, OUT], MM_DT)
                MH = MT1 // 2
                if s == 0:
                    # Two-pass accumulation, K-group outer: all of group 0's
                    # matmuls run while group 1's loads are still in flight;
                    # partials stash in SBUF and fold back in during group 1.
                    w_part = w_pool.tile(
                        [P, KT, OUT], mybir.dt.float32, name="w_part", bufs=1
                    )
                    for g in range(2):
                        for half in range(2):
                            psums = [
                                [
                                    psum_pool.tile(
                                        [P, NW],
                                        mybir.dt.float32,
                                        tag="ps",
                                        name=f"ps{ml}_{n}",
                                    )
                                    for n in range(NT)
                                ]
                                for ml in range(MH)
                            ]
                            for kl in range(KH):
                                for ml in range(MH):
                                    m = half * MH + ml
                                    for n in range(NT):
                                        nc.tensor.matmul(
                                            psums[ml][n][:],
                                            x_kg[g][:, kl, m * P : (m + 1) * P],
                                            mt_kg[g][:, kl, n * NW : (n + 1) * NW],
                                            start=(kl == 0),
                                            stop=(kl == KH - 1),
                                        )
                            for ml in range(MH):
                                m = half * MH + ml
                                for n in range(NT):
                                    dst = slice(n * NW, (n + 1) * NW)
                                    if g == 0:
                                        nc.vector.tensor_copy(
                                            w_part[:, m, dst], psums[ml][n][:]
                                        )
                                    else:
                                        nc.vector.tensor_add(
                                            out=w_sb[:, m, dst],
                                            in0=psums[ml][n][:],
                                            in1=w_part[:, m, dst],
                                        )
                else:
                    for half in range(2):
                        psums = [
                            [
                                psum_pool.tile(
                                    [P, NW],
                                    mybir.dt.float32,
                                    tag="ps",
                                    name=f"ps{ml}_{n}",
                                )
                                for n in range(NT)
                            ]
                            for ml in range(MH)
                        ]
                        for k in range(KT):
                            g, kl = divmod(k, KH)
                            for ml in range(MH):
                                m = half * MH + ml
                                for n in range(NT):
                                    nc.tensor.matmul(
                                        psums[ml][n][:],
                                        x_kg[g][:, kl, m * P : (m + 1) * P],
                                        mt_kg[g][:, kl, n * NW : (n + 1) * NW],
                                        start=(k == 0),
                                        stop=(k == KT - 1),
                                    )
                        for ml in range(MH):
                            m = half * MH + ml
                            for n in range(NT):
                                nc.vector.tensor_copy(
                                    w_sb[:, m, n * NW : (n + 1) * NW], psums[ml][n][:]
                                )
                w_r = w_sb[:]

                # Step 2: out = W^T @ Mt
                for m in range(MT2):
                    psums = [
                        psum_pool.tile([P, NW], mybir.dt.float32, tag="ps", name=f"ps{n}")
                        for n in range(NT)
                    ]
                    o_sb = o_pool.tile([M2, OUT], mybir.dt.float32)
                    for k in range(KT):
                        g, kl = divmod(k, KH)
                        for n in range(NT):
                            nc.tensor.matmul(
                                psums[n][:M2, :],
                                w_r[:, k, m * M2 : (m + 1) * M2],
                                mt_kg[g][:, kl, n * NW : (n + 1) * NW],
                                start=(k == 0),
                                stop=(k == KT - 1),
                            )
                    # Finer evict/DMA chunks on the very last group shorten the
                    # kernel tail (the final DMA starts sooner).
                    ev_chunks = 4 if (s == SLICES - 1 and m == MT2 - 1) else 1
                    for n in range(NT):
                        cw = NW // ev_chunks
                        for c in range(ev_chunks):
                            lo = n * NW + c * cw
                            nc.vector.tensor_copy(
                                o_sb[:, lo : lo + cw], psums[n][:M2, c * cw : (c + 1) * cw]
                            )
                            nc.sync.dma_start(
                                out_ext[s, m * M2 : (m + 1) * M2, lo : lo + cw],
                                o_sb[:, lo : lo + cw],
                            )

    nc.compile()
    return nc


_CACHE: dict = {}


def _get_program():
    if "nc" not in _CACHE:
        _CACHE["nc"] = _build_program()
        _CACHE["mt"] = _build_mt()
    return _CACHE["nc"], _CACHE["mt"]


def kernel(x: np.ndarray, _trace: bool = False):
    assert x.shape == (B, C, H, H), x.shape
    nc, mt = _get_program()
    x = np.ascontiguousarray(x, dtype=np.float32)
    # Pre-stripe on host: rows -> (ko, p) partitions, contiguous per partition.
    mt_arr = np.ascontiguousarray(
        mt.reshape(KT, P, OUT).transpose(1, 0, 2).reshape(P, KT * OUT)
    )
    x_arr = np.ascontiguousarray(
        x.reshape(B * C, KT, P, H).transpose(0, 2, 1, 3).reshape(B * C, P, KT * H)
    )
    per_core = B // N_CORES
    in_maps = [
        {
            "x": x_arr[i * SLICES : (i + 1) * SLICES],
            "mt": mt_arr,
        }
        for i in range(N_CORES)
    ]
    res = run_bass_kernel_spmd(nc, in_maps, list(range(N_CORES)), trace=_trace)
    out = np.empty((B, C, OUT, OUT), dtype=np.float32)
    for i in range(N_CORES):
        out[i * per_core : (i + 1) * per_core] = res.results[i]["out"].reshape(
            per_core, C, OUT, OUT
        )
    if _trace:
        return out, res
    return out


# revision 26
# speedup vs baseline: 1.0651x; 1.0125x over previous
"""DCT-based 1.25x upsample (2D DCT-II -> zero-pad spectrum -> 2D IDCT).

The whole reference computation is linear per (b, c) slice:
    out = M @ x @ M^T,   M = E960[:, :768] @ D768  (960x768, precomputed on host)
where D is the DCT-II matrix (norm=None) and E the IDCT matrix; zero-padding
the spectrum is folded into taking the first 768 columns of E.

On each NeuronCore (data-parallel over batch: 16 batches / 8 cores = 2 each,
x 3 channels = 6 slices per core) we run two chained matmuls per slice with
the tensor engine's `out = lhsT.T @ rhs` convention and the shared constant
Mt = M^T (768x960) as the moving operand:
    step 1:  W   = matmul(lhsT=x,  rhs=Mt) = x^T @ M^T         [768, 960]
    step 2:  out = matmul(lhsT=W,  rhs=Mt) = M @ x @ M^T       [960, 960]
W comes out of step 1 in PSUM with exactly the [K-partition, free] layout
step 2 needs for lhsT, so no transposes anywhere.

Matmuls run as float32r (fp32 bits, FP22 multiplies, fp32 accumulate):
1 PE cycle/row vs 4 for true fp32, end-to-end rel err ~1e-4.
"""

import numpy as np

import concourse.bass as bass  # noqa: F401  (engine types route via nc)
import concourse.mybir as mybir
import concourse.tile as tile
from concourse import bacc
from concourse.bass_utils import run_bass_kernel_spmd

# Problem shape (hardcoded per contract)
B, C, H = 16, 3, 768
OUT = 960  # H * 1.25
N_CORES = 8
SLICES = (B * C) // N_CORES  # 6 per core

P = 128
KT = H // P          # 6 contraction tiles
MT1 = H // P         # 6 output-row tiles for step 1 (x columns)
M2 = 120
MT2 = OUT // M2      # 8 output-row tiles for step 2
NT = 2
NW = OUT // NT       # 480-wide moving chunks (<= 512 fp32 PSUM bank)

MM_DT = mybir.dt.float32r  # set to mybir.dt.float32 for full-precision (4x slower)


def _build_mt() -> np.ndarray:
    """Mt = (E960[:, :768] @ D768)^T as float32, computed in float64."""
    n = np.arange(H, dtype=np.float64)
    k = np.arange(H, dtype=np.float64)[:, None]
    D = 2.0 * np.cos(np.pi * (2.0 * n[None, :] + 1.0) * k / (2.0 * H))

    n2 = np.arange(OUT, dtype=np.float64)[:, None]
    k2 = np.arange(OUT, dtype=np.float64)[None, :]
    E = np.cos(np.pi * (2.0 * n2 + 1.0) * k2 / (2.0 * OUT)) / OUT
    E[:, 0] = 1.0 / (2.0 * OUT)

    M = E[:, :H] @ D  # [960, 768]
    return np.ascontiguousarray(M.T).astype(np.float32)  # [768, 960]


def _build_program():
    nc = bacc.Bacc(None, target_bir_lowering=False, debug=False)

    # Both inputs are pre-arranged on the host into the striped SBUF layout
    # (partition-major), so every load DMA is one contiguous run per partition.
    x_ext = nc.dram_tensor("x", [SLICES, P, KT * H], MM_DT, kind="ExternalInput")
    mt_ext = nc.dram_tensor("mt", [P, KT * OUT], MM_DT, kind="ExternalInput")
    out_ext = nc.dram_tensor(
        "out", [SLICES, OUT, OUT], mybir.dt.float32, kind="ExternalOutput"
    )

    with tile.TileContext(nc) as tc:
        with (
            tc.tile_pool(name="const", bufs=1) as const_pool,
            tc.tile_pool(name="xp", bufs=4) as x_pool,
            tc.tile_pool(name="wp", bufs=2) as w_pool,
            tc.tile_pool(name="op", bufs=4) as o_pool,
            tc.tile_pool(name="ps", bufs=8, space="PSUM") as psum_pool,
        ):
            # PE warmup: ~16 dummy matmuls on memset tiles keep the tensor
            # engine busy while the first loads land, so the HAM clock gate is
            # already at 2.4 GHz when real matmuls start.
            warm_f32 = const_pool.tile([P, NW], mybir.dt.float32, name="warm_f32")
            nc.gpsimd.memset(warm_f32[:], 0.0)
            warm_w = const_pool.tile([P, P], MM_DT, name="warm_w")
            warm_m = const_pool.tile([P, NW], MM_DT, name="warm_m")
            nc.vector.tensor_copy(warm_w[:], warm_f32[:, :P])
            nc.scalar.copy(warm_m[:], warm_f32[:])
            warm_ps = psum_pool.tile([P, NW], mybir.dt.float32, tag="ps", name="warm_ps")
            for _ in range(20):
                nc.tensor.matmul(warm_ps[:], warm_w[:], warm_m[:], start=True, stop=True)

            # Shared constant Mt split into two K-group tiles (contiguous DRAM
            # runs per partition); the first matmuls only wait on group 0.
            # mt_kg[g][p, kl, n] = Mt[(g*KH + kl)*P + p, n]
            KH = KT // 2
            mt_dram = mt_ext[:].rearrange("p (ko n) -> p ko n", n=OUT)
            mt_kg = [
                const_pool.tile([P, KH, OUT], MM_DT, name=f"mt{g}") for g in range(2)
            ]
            nc.sync.dma_start(mt_kg[0][:], mt_dram[:, 0:KH, :])

            for s in range(SLICES):
                # x slice split into two K-group (row) tiles:
                # x_kg[g][p, kl, j] = x[(g*KH + kl)*P + p, j]
                x_dram = x_ext[s].rearrange("p (ko j) -> p ko j", j=H)
                x_kg = []
                for g in range(2):
                    xg = x_pool.tile([P, KH, H], MM_DT, tag="x", name=f"x{g}")
                    nc.sync.dma_start(xg[:], x_dram[:, g * KH : (g + 1) * KH, :])
                    x_kg.append(xg)
                    if s == 0 and g == 0:
                        nc.sync.dma_start(mt_kg[1][:], mt_dram[:, KH:KT, :])

                # Step 1: W = x^T @ Mt, K-striped for step 2:
                # w_sb[p, m, l] = W[m*P + p, l]
                # k-outer over halves of m so first-slice matmuls start as soon
                # as stripe k=0 lands (6 live PSUM banks per half).
                w_sb = w_pool.tile([P, KT, OUT], MM_DT)
                MH = MT1 // 2
                if s == 0:
                    # Two-pass accumulation, K-group outer: all of group 0's
                    # matmuls run while group 1's loads are still in flight;
                    # partials stash in SBUF and fold back in during group 1.
                    w_part = w_pool.tile(
                        [P, KT, OUT], mybir.dt.float32, name="w_part", bufs=1
                    )
                    for g in range(2):
                        for half in range(2):
                            psums = [
                                [
                                    psum_pool.tile(
                                        [P, NW],
                                        mybir.dt.float32,
                                        tag="ps",
                                        name=f"ps{ml}_{n}",
                                    )
                                    for n in range(NT)
                                ]
                                for ml in range(MH)
                            ]
                            for kl in range(KH):
                                for ml in range(MH):
                                    m = half * MH + ml
                                    for n in range(NT):
                                        nc.tensor.matmul(
                                            psums[ml][n][:],
                                            x_kg[g][:, kl, m * P : (m + 1) * P],
                                            mt_kg[g][:, kl, n * NW : (n + 1) * NW],
                                            start=(kl == 0),
                                            stop=(kl == KH - 1),
                                        )
                            for ml in range(MH):
                                m = half * MH + ml
                                for n in range(NT):
                                    dst = slice(n * NW, (n + 1) * NW)
                                    if g == 0:
                                        nc.vector.tensor_copy(
                                            w_part[:, m, dst], psums[ml][n][:]
                                        )
                                    else:
                                        nc.vector.tensor_add(
                                            out=w_sb[:, m, dst],
                                            in0=psums[ml][n][:],
                                            in1=w_part[:, m, dst],
                                        )
                else:
                    for half in range(2):
                        psums = [
                            [
                                psum_pool.tile(
                                    [P, NW],
                                    mybir.dt.float32,
                                    tag="ps",
                                    name=f"ps{ml}_{n}",
                                )
                                for n in range(NT)
                            ]
                            for ml in range(MH)
                        ]
                        for k in range(KT):
                            g, kl = divmod(k, KH)
                            for ml in range(MH):
                                m = half * MH + ml
                                for n in range(NT):
                                    nc.tensor.matmul(
                                        psums[ml][n][:],
                                        x_kg[g][:, kl, m * P : (m + 1) * P],
                                        mt_kg[g][:, kl, n * NW : (n + 1) * NW],
                                        start=(k == 0),
                                        stop=(k == KT - 1),
                                    )
                        for ml in range(MH):
                            m = half * MH + ml
                            for n in range(NT):
                                nc.vector.tensor_copy(
                                    w_sb[:, m, n * NW : (n + 1) * NW], psums[ml][n][:]
                                )
                w_r = w_sb[:]

                # Step 2: out = W^T @ Mt
                for m in range(MT2):
                    psums = [
                        psum_pool.tile([P, NW], mybir.dt.float32, tag="ps", name=f"ps{n}")
                        for n in range(NT)
                    ]
                    o_sb = o_pool.tile([M2, OUT], mybir.dt.float32)
                    for k in range(KT):
                        g, kl = divmod(k, KH)
                        for n in range(NT):
                            nc.tensor.matmul(
                                psums[n][:M2, :],
                                w_r[:, k, m * M2 : (m + 1) * M2],
                                mt_kg[g][:, kl, n * NW : (n + 1) * NW],
                                start=(k == 0),
                                stop=(k == KT - 1),
                            )
                    for n in range(NT):
                        nc.vector.tensor_copy(
                            o_sb[:, n * NW : (n + 1) * NW], psums[n][:M2, :]
                        )
                        nc.sync.dma_start(
                            out_ext[s, m * M2 : (m + 1) * M2, n * NW : (n + 1) * NW],
                            o_sb[:, n * NW : (n + 1) * NW],
                        )

    nc.compile()
    return nc


_CACHE: dict = {}


def _get_program():
    if "nc" not in _CACHE:
        _CACHE["nc"] = _build_program()
        _CACHE["mt"] = _build_mt()
    return _CACHE["nc"], _CACHE["mt"]


def kernel(x: np.ndarray, _trace: bool = False):
    assert x.shape == (B, C, H, H), x.shape
    nc, mt = _get_program()
    x = np.ascontiguousarray(x, dtype=np.float32)
    # Pre-stripe on host: rows -> (ko, p) partitions, contiguous per partition.
    mt_arr = np.ascontiguousarray(
        mt.reshape(KT, P, OUT).transpose(1, 0, 2).reshape(P, KT * OUT)
    )
    x_arr = np.ascontiguousarray(
        x.reshape(B * C, KT, P, H).transpose(0, 2, 1, 3).reshape(B * C, P, KT * H)
    )
    per_core = B // N_CORES
    in_maps = [
        {
            "x": x_arr[i * SLICES : (i + 1) * SLICES],
            "mt": mt_arr,
        }
        for i in range(N_CORES)
    ]
    res = run_bass_kernel_spmd(nc, in_maps, list(range(N_CORES)), trace=_trace)
    out = np.empty((B, C, OUT, OUT), dtype=np.float32)
    for i in range(N_CORES):
        out[i * per_core : (i + 1) * per_core] = res.results[i]["out"].reshape(
            per_core, C, OUT, OUT
        )
    if _trace:
        return out, res
    return out
